# revision 11
# baseline (speedup 1.0000x reference)
"""Trainium2 Bass kernel for nn_ConvLayer_51771535786262 (GNN message passing).

  edge_input = [x[row], x[col], edge_attr]            # [E, 384]
  h   = softplus(edge_input @ W1 + b1)                # [E, 256]
  emb = softplus(h @ W2 + b2)                         # [E, 128]
  aggr = segment_sum(emb, col, N)                     # [N, 128]
  out = softplus([x, aggr] @ Wn + bn) + x             # [N, 128]

Strategy: sort edges by destination node block (col // 128); assign 49
consecutive node blocks (6272 nodes) to each of the 8 cores so every edge's
scatter target is core-local (no cross-core communication). Each per-block
edge group is padded to a uniform G edges so all cores run one SPMD program.

Key kernel structure (per core, per group of G edges):
- x[row] features are pre-gathered per edge slot on the host and
  streamed feature-major in bf16 (sequential DMA; no on-device gather,
  no PE transposes).
- x[col] is block-local (col // 128 == block id), so its layer-1
  contribution uses host-precomputed xW1b = x_block @ W1b selected by a
  one-hot matrix S'[n, e] = (col_local[e] == n) built on-chip (rank-1
  broadcast matmul + vector is_equal).
- Layer 1 runs feature-major, weight-stationary, with b1 fused into the
  exp activation; softplus = ln(1+exp(.)) with both exp and ln drawn from
  the single natural_log_exp_and_others table set (one table load total).
- Layer 2 runs data-stationary (edge-major out) with b2/bn added via
  vector-engine adds of host-broadcast bias tiles; scatter accumulates
  one-hot matmuls into PSUM per node block.
- Node MLP interleaved into the group loop every 4 blocks (fp32).
"""

import sys

sys.path.insert(0, "/opt/trn_rl_repo")

import numpy as np
import ml_dtypes

import concourse.bass as bass
import concourse.mybir as mybir
import concourse.tile as tile
import bass_rust
from concourse import bacc
from concourse.bass_utils import run_bass_kernel_spmd
from concourse.hw_specs import get_activation_tables

BF16 = mybir.dt.bfloat16
F32 = mybir.dt.float32
I32 = mybir.dt.int32
I16 = mybir.dt.int16
AF = mybir.ActivationFunctionType

N_NODES = 50000
N_EDGES = 600000
D = 128
N_CORES = 8
NBLK = 49            # node blocks per core
NPC = NBLK * D       # 6272 nodes per core
TAB = 32768          # gather table rows (int16-addressable)
GSPLIT = 25          # groups 0..24 use table A, 25..48 table B


def _subs(nch):
    """Split nch chunks into pieces of <=4 chunks."""
    sizes = []
    left = nch
    while left > 0:
        take = min(4, left)
        sizes.append(take)
        left -= take
    return sizes


def _pieces(nch):
    """Split nch chunks into gather pieces of <=7 chunks (896 idxs)."""
    n = (nch + 6) // 7
    base = nch // n
    sizes = [base + (1 if i < nch % n else 0) for i in range(n)]
    return sizes


def build_program(ctx, tc, aps, nblk, nch):
    nc = tc.nc
    G = nch * D
    GI = G // 16
    subs = _subs(nch)

    consts = ctx.enter_context(tc.tile_pool(name="consts", bufs=1))
    sb = ctx.enter_context(tc.tile_pool(name="sb", bufs=3))
    sbL = ctx.enter_context(tc.tile_pool(name="sbL", bufs=2))
    sbn = ctx.enter_context(tc.tile_pool(name="sbn", bufs=2))
    pp_pre = ctx.enter_context(tc.tile_pool(name="pp_pre", bufs=3, space="PSUM"))
    pp_cl = ctx.enter_context(tc.tile_pool(name="pp_cl", bufs=2, space="PSUM"))
    pp_b = ctx.enter_context(tc.tile_pool(name="pp_b", bufs=2, space="PSUM"))
    pp_g = ctx.enter_context(tc.tile_pool(name="pp_g", bufs=1, space="PSUM"))

    # single activation table load: natural_log_exp_and_others has exp+ln
    set_id = list(get_activation_tables(nc.m.arch)).index(
        "natural_log_exp_and_others")
    nc.scalar.add_instruction(bass_rust.InstLoadActFuncSet(
        act_func_set_id=set_id,
        name=nc.get_next_instruction_name(),
        engine=mybir.EngineType.Activation,
    ))

    # ---- constants ----
    iota_i = consts.tile([D, D], I32)
    nc.gpsimd.iota(iota_i[:], pattern=[[1, D]], base=0, channel_multiplier=0)
    iota_b = consts.tile([D, D], BF16)
    nc.vector.tensor_copy(iota_b[:], iota_i[:])

    iotap_i = consts.tile([D, 1], I32)
    nc.gpsimd.iota(iotap_i[:], pattern=[[0, 1]], base=0, channel_multiplier=1)
    iota_p = consts.tile([D, 1], F32)
    nc.vector.tensor_copy(iota_p[:], iotap_i[:])

    ones_b = consts.tile([1, D], BF16)
    nc.gpsimd.memset(ones_b[:], 1.0)
    ones_f = consts.tile([1, D], F32)
    nc.gpsimd.memset(ones_f[:], 1.0)

    w1a = consts.tile([D, 256], BF16)
    nc.sync.dma_start(w1a[:], aps["w1a"][:])
    w1c = consts.tile([D, 256], BF16)
    nc.sync.dma_start(w1c[:], aps["w1c"][:])
    b1c = consts.tile([D, 2], F32)
    nc.sync.dma_start(b1c[:], aps["b1c"][:])
    w2_0 = consts.tile([D, D], BF16)
    nc.sync.dma_start(w2_0[:], aps["w2"][0:D, :])
    w2_1 = consts.tile([D, D], BF16)
    nc.sync.dma_start(w2_1[:], aps["w2"][D: 2 * D, :])
    b2r = consts.tile([1, D], BF16)
    nc.sync.dma_start(b2r[:], aps["b2r"][:])
    wn_x = consts.tile([D, D], F32)
    nc.sync.dma_start(wn_x[:], aps["wn"][0:D, :])
    wn_a = consts.tile([D, D], F32)
    nc.sync.dma_start(wn_a[:], aps["wn"][D: 2 * D, :])
    bnr = consts.tile([1, D], F32)
    nc.sync.dma_start(bnr[:], aps["bnr"][:])

    cl_t = consts.tile([D, nblk * nch], BF16)
    nc.sync.dma_start(cl_t[:], aps["cl"][:])

    # persistent per-core tensors
    xt_t = consts.tile([D, nblk * D], F32)
    nc.sync.dma_start(xt_t[:], aps["xt"][:])
    aggrT = consts.tile([D, nblk * D], F32)

    b2bc = consts.tile([D, 512], F32)
    nc.sync.dma_start(b2bc[:], aps["b2bc"][:])
    bnbc = consts.tile([D, 512], F32)
    nc.sync.dma_start(bnbc[:], aps["bnbc"][:])

    ea_dram = aps["ea"]
    xr_dram = aps["xr"]
    clr_dram = aps["clr"]
    xw_dram = aps["xw1b"]
    xb_dram = aps["xb"]
    out_dram = aps["out"]

    for g in range(nblk):
        # ---- loads ----
        ea_t = sb.tile([D, G], BF16, tag="ea")
        nc.sync.dma_start(ea_t[:], ea_dram[:, g * G: (g + 1) * G])
        clr_t = sb.tile([1, G], BF16, tag="clr")
        nc.sync.dma_start(clr_t[:], clr_dram[:, g * G: (g + 1) * G])
        xw_t = sb.tile([D, 256], BF16, tag="xw")
        nc.sync.dma_start(xw_t[:], xw_dram[g * D: (g + 1) * D, :])

        # ---- x[row] features, pre-gathered on host, streamed bf16 ----
        xrT = sb.tile([D, G], BF16, tag="xrT")
        nc.sync.dma_start(xrT[:], xr_dram[:, g * G: (g + 1) * G])

        # ---- S' one-hot [node, edge] for the x[col] term ----
        sprime = sb.tile([D, G], BF16, tag="sp")
        off = 0
        for ns in subs:
            L = ns * D
            clp = pp_cl.tile([D, 512], F32, space="PSUM", tag="clp")
            nc.tensor.matmul(clp[:, 0:L], lhsT=ones_b[:],
                             rhs=clr_t[:, off: off + L], start=True, stop=True)
            nc.vector.tensor_tensor(
                out=sprime[:, off: off + L],
                in0=clp[:, 0:L],
                in1=iota_p[:].to_broadcast([D, L]),
                op=mybir.AluOpType.is_equal,
            )
            off += L

        # ---- layer 1 (feature-major), u = exp(pre + b1) in bf16 ----
        u_t = sbL.tile([D, 2 * G], BF16, tag="u")
        off = 0
        for ns in subs:
            L = ns * D
            for m in range(2):
                ms = slice(m * D, (m + 1) * D)
                pre = pp_pre.tile([D, 512], F32, space="PSUM", tag="pre")
                nc.tensor.matmul(pre[:, 0:L], lhsT=w1a[:, ms],
                                 rhs=xrT[:, off: off + L], start=True, stop=False)
                nc.tensor.matmul(pre[:, 0:L], lhsT=xw_t[:, ms],
                                 rhs=sprime[:, off: off + L], start=False, stop=False)
                nc.tensor.matmul(pre[:, 0:L], lhsT=w1c[:, ms],
                                 rhs=ea_t[:, off: off + L], start=False, stop=True)
                nc.scalar.activation(
                    u_t[:, m * G + off: m * G + off + L], pre[:, 0:L],
                    AF.Exp, bias=b1c[:, m: m + 1],
                )
            off += L
        # hT = ln(1 + u), one instruction per half (pipeline smoothing)
        hT = sbL.tile([D, 2 * G], BF16, tag="hT")
        nc.scalar.activation(hT[:, 0:G], u_t[:, 0:G], AF.Ln, bias=1.0)
        nc.scalar.activation(hT[:, G: 2 * G], u_t[:, G: 2 * G], AF.Ln, bias=1.0)

        # ---- layer 2 (data-stationary, edge-major out) + scatter ----
        u2 = sb.tile([D, G], BF16, tag="u2")
        v2 = sbL.tile([D, G], F32, tag="v2")
        c0 = 0
        for ns in subs:
            eps = pp_b.tile([D, 512], F32, space="PSUM", tag="eps")
            for i in range(ns):
                c = c0 + i
                es = slice(i * D, (i + 1) * D)
                nc.tensor.matmul(eps[:, es], lhsT=hT[:, c * D: (c + 1) * D],
                                 rhs=w2_0[:], start=True, stop=False)
                nc.tensor.matmul(eps[:, es],
                                 lhsT=hT[:, G + c * D: G + (c + 1) * D],
                                 rhs=w2_1[:], start=False, stop=True)
            nc.vector.tensor_add(v2[:, c0 * D: (c0 + ns) * D],
                                 eps[:, 0: ns * D], b2bc[:, 0: ns * D])
            c0 += ns
        nc.scalar.activation(u2[:], v2[:], AF.Exp)
        embs = sb.tile([D, G], BF16, tag="embs")
        nc.scalar.activation(embs[:], u2[:], AF.Ln, bias=1.0)

        agg = pp_g.tile([D, D], F32, space="PSUM", tag="agg")
        for c in range(nch):
            S_t = sb.tile([D, D], BF16, tag="S")
            nc.vector.tensor_tensor(
                out=S_t[:],
                in0=cl_t[:, g * nch + c: g * nch + c + 1].to_broadcast([D, D]),
                in1=iota_b[:],
                op=mybir.AluOpType.is_equal,
            )
            nc.tensor.matmul(agg[:], lhsT=embs[:, c * D: (c + 1) * D], rhs=S_t[:],
                             start=(c == 0), stop=(c == nch - 1))
        nc.vector.tensor_copy(aggrT[:, g * D: (g + 1) * D], agg[:])

        # ---- node MLP for finished blocks, every 4 groups (fp32) ----
        if g % 4 == 3 or g == nblk - 1:
            j0 = (g // 4) * 4
            nset = g + 1 - j0
            W = nset * D
            yps = pp_b.tile([D, 512], F32, space="PSUM", tag="eps")
            for i in range(nset):
                j = j0 + i
                ys = slice(i * D, (i + 1) * D)
                nc.tensor.matmul(yps[:, ys], lhsT=xt_t[:, j * D: (j + 1) * D],
                                 rhs=wn_x[:], start=True, stop=False)
                nc.tensor.matmul(yps[:, ys], lhsT=aggrT[:, j * D: (j + 1) * D],
                                 rhs=wn_a[:], start=False, stop=True)
            vy = sbn.tile([D, 512], F32, tag="vy")
            nc.vector.tensor_add(vy[:, 0:W], yps[:, 0:W], bnbc[:, 0:W])
            uy = sbn.tile([D, 512], F32, tag="uy")
            nc.scalar.activation(uy[:, 0:W], vy[:, 0:W], AF.Exp)
            sp = sbn.tile([D, 512], F32, tag="spn")
            nc.scalar.activation(sp[:, 0:W], uy[:, 0:W], AF.Ln, bias=1.0)
            xb_t = sbn.tile([D, 512], F32, tag="xb")
            nc.sync.dma_start(
                xb_t[:, 0:W].rearrange("p (c f) -> p c f", f=D),
                xb_dram[j0 * D: j0 * D + W, :].rearrange("(c p) f -> p c f", p=D),
            )
            ot = sbn.tile([D, 512], F32, tag="ot")
            nc.vector.tensor_add(ot[:, 0:W], sp[:, 0:W], xb_t[:, 0:W])
            nc.sync.dma_start(
                out_dram[j0 * D: j0 * D + W, :].rearrange("(c p) f -> p c f", p=D),
                ot[:, 0:W].rearrange("p (c f) -> p c f", f=D),
            )


def build_nc(nblk, nch, num_devices=1):
    nc = bacc.Bacc("TRN2", target_bir_lowering=False, debug=False,
                   num_devices=num_devices)
    G = nch * D
    GI = G // 16
    specs = {
        "xr": ([D, nblk * G], BF16),
        "b2bc": ([D, 512], F32),
        "bnbc": ([D, 512], F32),
        "xt": ([D, nblk * D], F32),
        "xb": ([nblk * D, D], F32),
        "ea": ([D, nblk * G], BF16),
        "clr": ([1, nblk * G], BF16),
        "cl": ([D, nblk * nch], BF16),
        "xw1b": ([nblk * D, 256], BF16),
        "w1a": ([D, 256], BF16),
        "w1c": ([D, 256], BF16),
        "b1c": ([D, 2], F32),
        "w2": ([256, D], BF16),
        "b2r": ([1, D], BF16),
        "wn": ([256, D], F32),
        "bnr": ([1, D], F32),
    }
    aps = {}
    for name, (shape, dt) in specs.items():
        aps[name] = nc.dram_tensor(name, shape, dt, kind="ExternalInput").ap()
    aps["out"] = nc.dram_tensor("out", [nblk * D, D], F32,
                                kind="ExternalOutput").ap()

    from contextlib import ExitStack

    with tile.TileContext(nc) as tc, ExitStack() as ctx:
        build_program(ctx, tc, aps, nblk, nch)
    nc.compile()
    return nc


def host_prep(x, edge_index, edge_attr, W1, b1, W2, b2, Wn, bn,
              n_nodes, n_cores, nblk):
    bf = ml_dtypes.bfloat16
    npc = nblk * D
    n_blocks_tot = n_cores * nblk

    row = np.asarray(edge_index[0], dtype=np.int64)
    col = np.asarray(edge_index[1], dtype=np.int64)
    E = row.shape[0]
    B = col // D
    order = np.argsort(B, kind="stable")
    counts = np.bincount(B, minlength=n_blocks_tot)
    G = int(np.ceil(max(int(counts.max()), 256) / D) * D)
    nch = G // D

    starts = np.zeros(n_blocks_tot, dtype=np.int64)
    starts[1:] = np.cumsum(counts)[:-1]
    pos = np.arange(E, dtype=np.int64) - starts[B[order]]
    slot = B[order] * G + pos            # slot in flat padded edge array

    flat_row = np.full(n_blocks_tot * G, -1, dtype=np.int64)  # -1 = padding
    flat_row[slot] = row[order]
    flat_cl = np.full(n_blocks_tot * G, 300.0, dtype=np.float32)
    flat_cl[slot] = (col[order] % D).astype(np.float32)
    flat_ea = np.zeros((n_blocks_tot * G, D), dtype=bf)
    flat_ea[slot] = edge_attr[order].astype(bf)

    x32 = np.ascontiguousarray(x).astype(np.float32)
    x_bf = x32.astype(bf)

    w1a = np.ascontiguousarray(W1[0:D]).astype(bf)
    w1b32 = np.ascontiguousarray(W1[D: 2 * D]).astype(np.float32)
    w1c = np.ascontiguousarray(W1[2 * D: 3 * D]).astype(bf)
    b1c = np.ascontiguousarray(np.asarray(b1).reshape(2, D).T).astype(np.float32)
    w2 = np.ascontiguousarray(W2).astype(bf)
    b2r = np.ascontiguousarray(np.asarray(b2)[None, :]).astype(bf)
    wn = np.ascontiguousarray(Wn).astype(np.float32)
    bnr = np.ascontiguousarray(np.asarray(bn)[None, :]).astype(np.float32)

    GI = G // 16
    in_maps = []
    for k in range(n_cores):
        lo, hi = k * npc, min((k + 1) * npc, n_nodes)
        xk = np.zeros((npc, D), dtype=np.float32)
        xk[0: hi - lo] = x32[lo:hi]

        rows_k = flat_row[k * nblk * G: (k + 1) * nblk * G]  # [nblk*G]
        xr_rows = np.zeros((nblk * G, D), dtype=bf)
        real = rows_k >= 0
        xr_rows[real] = x_bf[rows_k[real]]
        xr_k = np.ascontiguousarray(xr_rows.T)

        ea_k = np.ascontiguousarray(
            flat_ea[k * nblk * G: (k + 1) * nblk * G].T)
        cl_k = flat_cl[k * nblk * G: (k + 1) * nblk * G]
        clr = np.ascontiguousarray(cl_k[None, :]).astype(bf)
        cl_sw = np.ascontiguousarray(
            cl_k.reshape(nblk, nch, D).transpose(2, 0, 1).reshape(D, nblk * nch)
        ).astype(bf)
        xw1b = (xk @ w1b32).astype(bf)   # [npc, 256]

        in_maps.append({
            "xr": xr_k,
            "b2bc": np.tile(np.asarray(b2, np.float32)[None, :], (D, 4)),
            "bnbc": np.tile(np.asarray(bn, np.float32)[None, :], (D, 4)),
            "xt": np.ascontiguousarray(xk.T), "xb": xk,
            "ea": ea_k, "clr": clr, "cl": cl_sw,
            "xw1b": np.ascontiguousarray(xw1b),
            "w1a": w1a, "w1c": w1c, "b1c": b1c,
            "w2": w2, "b2r": b2r, "wn": wn, "bnr": bnr,
        })
    return in_maps, nch


def run(inputs, trace=False, **kw):
    in_maps, nch = host_prep(
        inputs["x"], inputs["edge_index"], inputs["edge_attr"],
        inputs["W1"], inputs["b1"], inputs["W2"], inputs["b2"],
        inputs["Wn"], inputs["bn"],
        n_nodes=N_NODES, n_cores=N_CORES, nblk=NBLK,
    )
    nc = build_nc(NBLK, nch, num_devices=N_CORES)
    res = run_bass_kernel_spmd(nc, in_maps, core_ids=list(range(N_CORES)),
                               trace=trace, **kw)
    out = np.concatenate([res.results[k]["out"] for k in range(N_CORES)], axis=0)
    return out[:N_NODES], res


def kernel(**inputs) -> np.ndarray:
    out, _ = run(inputs, trace=False)
    return np.ascontiguousarray(out.astype(np.float32))


# revision 12
# speedup vs baseline: 1.0424x; 1.0424x over previous
"""Trainium2 Bass kernel for nn_ConvLayer_51771535786262 (GNN message passing).

  edge_input = [x[row], x[col], edge_attr]            # [E, 384]
  h   = softplus(edge_input @ W1 + b1)                # [E, 256]
  emb = softplus(h @ W2 + b2)                         # [E, 128]
  aggr = segment_sum(emb, col, N)                     # [N, 128]
  out = softplus([x, aggr] @ Wn + bn) + x             # [N, 128]

Strategy: sort edges by destination node block (col // 128); assign 49
consecutive node blocks (6272 nodes) to each of the 8 cores so every edge's
scatter target is core-local (no cross-core communication). Each per-block
edge group is padded to a uniform G edges so all cores run one SPMD program.

Key kernel structure (per core, per group of G edges):
- x[row] features are pre-gathered per edge slot on the host and
  streamed feature-major in bf16 (sequential DMA; no on-device gather,
  no PE transposes).
- x[col] is block-local (col // 128 == block id), so its layer-1
  contribution uses host-precomputed xW1b = x_block @ W1b selected by a
  one-hot matrix S'[n, e] = (col_local[e] == n) built on-chip (rank-1
  broadcast matmul + vector is_equal).
- Layer 1 runs feature-major, weight-stationary, with b1 fused into the
  exp activation; softplus = ln(1+exp(.)) with both exp and ln drawn from
  the single natural_log_exp_and_others table set (one table load total).
- Layer 2 runs data-stationary (edge-major out) with b2/bn added via
  vector-engine adds of host-broadcast bias tiles; scatter accumulates
  one-hot matmuls into PSUM per node block.
- Node MLP interleaved into the group loop every 4 blocks (fp32).
"""

import sys

sys.path.insert(0, "/opt/trn_rl_repo")

import numpy as np
import ml_dtypes

import concourse.bass as bass
import concourse.mybir as mybir
import concourse.tile as tile
import bass_rust
from concourse import bacc
from concourse.bass_utils import run_bass_kernel_spmd
from concourse.hw_specs import get_activation_tables

BF16 = mybir.dt.bfloat16
F32 = mybir.dt.float32
I32 = mybir.dt.int32
I16 = mybir.dt.int16
AF = mybir.ActivationFunctionType

N_NODES = 50000
N_EDGES = 600000
D = 128
N_CORES = 8
NBLK = 49            # node blocks per core
NPC = NBLK * D       # 6272 nodes per core
TAB = 32768          # gather table rows (int16-addressable)
GSPLIT = 25          # groups 0..24 use table A, 25..48 table B


def _subs(nch):
    """Split nch chunks into pieces of <=4 chunks."""
    sizes = []
    left = nch
    while left > 0:
        take = min(4, left)
        sizes.append(take)
        left -= take
    return sizes


def _pieces(nch):
    """Split nch chunks into gather pieces of <=7 chunks (896 idxs)."""
    n = (nch + 6) // 7
    base = nch // n
    sizes = [base + (1 if i < nch % n else 0) for i in range(n)]
    return sizes


def build_program(ctx, tc, aps, nblk, nch):
    nc = tc.nc
    G = nch * D
    GI = G // 16
    subs = _subs(nch)

    consts = ctx.enter_context(tc.tile_pool(name="consts", bufs=1))
    sb = ctx.enter_context(tc.tile_pool(name="sb", bufs=3))
    sbL = ctx.enter_context(tc.tile_pool(name="sbL", bufs=2))
    sbn = ctx.enter_context(tc.tile_pool(name="sbn", bufs=2))
    pp_pre = ctx.enter_context(tc.tile_pool(name="pp_pre", bufs=3, space="PSUM"))
    pp_cl = ctx.enter_context(tc.tile_pool(name="pp_cl", bufs=2, space="PSUM"))
    pp_b = ctx.enter_context(tc.tile_pool(name="pp_b", bufs=2, space="PSUM"))
    pp_g = ctx.enter_context(tc.tile_pool(name="pp_g", bufs=1, space="PSUM"))

    # single activation table load: natural_log_exp_and_others has exp+ln
    set_id = list(get_activation_tables(nc.m.arch)).index(
        "natural_log_exp_and_others")
    nc.scalar.add_instruction(bass_rust.InstLoadActFuncSet(
        act_func_set_id=set_id,
        name=nc.get_next_instruction_name(),
        engine=mybir.EngineType.Activation,
    ))

    # ---- constants ----
    iota_i = consts.tile([D, D], I32)
    nc.gpsimd.iota(iota_i[:], pattern=[[1, D]], base=0, channel_multiplier=0)
    iota_b = consts.tile([D, D], BF16)
    nc.vector.tensor_copy(iota_b[:], iota_i[:])

    iotap_i = consts.tile([D, 1], I32)
    nc.gpsimd.iota(iotap_i[:], pattern=[[0, 1]], base=0, channel_multiplier=1)
    iota_p = consts.tile([D, 1], F32)
    nc.vector.tensor_copy(iota_p[:], iotap_i[:])

    ones_b = consts.tile([1, D], BF16)
    nc.gpsimd.memset(ones_b[:], 1.0)
    ones_f = consts.tile([1, D], F32)
    nc.gpsimd.memset(ones_f[:], 1.0)

    w1a = consts.tile([D, 256], BF16)
    nc.sync.dma_start(w1a[:], aps["w1a"][:])
    w1c = consts.tile([D, 256], BF16)
    nc.sync.dma_start(w1c[:], aps["w1c"][:])
    b1c = consts.tile([D, 2], F32)
    nc.sync.dma_start(b1c[:], aps["b1c"][:])
    w2_0 = consts.tile([D, D], BF16)
    nc.sync.dma_start(w2_0[:], aps["w2"][0:D, :])
    w2_1 = consts.tile([D, D], BF16)
    nc.sync.dma_start(w2_1[:], aps["w2"][D: 2 * D, :])
    b2r = consts.tile([1, D], BF16)
    nc.sync.dma_start(b2r[:], aps["b2r"][:])
    wn_x = consts.tile([D, D], F32)
    nc.sync.dma_start(wn_x[:], aps["wn"][0:D, :])
    wn_a = consts.tile([D, D], F32)
    nc.sync.dma_start(wn_a[:], aps["wn"][D: 2 * D, :])
    bnr = consts.tile([1, D], F32)
    nc.sync.dma_start(bnr[:], aps["bnr"][:])

    cl_t = consts.tile([D, nblk * nch], BF16)
    nc.sync.dma_start(cl_t[:], aps["cl"][:])

    # persistent per-core tensors
    xt_t = consts.tile([D, nblk * D], F32)
    nc.sync.dma_start(xt_t[:], aps["xt"][:])
    aggrT = consts.tile([D, nblk * D], F32)

    b2bc = consts.tile([D, 512], F32)
    nc.sync.dma_start(b2bc[:], aps["b2bc"][:])
    bnbc = consts.tile([D, 512], F32)
    nc.sync.dma_start(bnbc[:], aps["bnbc"][:])

    ea_dram = aps["ea"]
    xr_dram = aps["xr"]
    clr_dram = aps["clr"]
    xw_dram = aps["xw1b"]
    xb_dram = aps["xb"]
    out_dram = aps["out"]

    for g in range(nblk):
        # ---- loads ----
        ea_t = sb.tile([D, G], BF16, tag="ea")
        nc.sync.dma_start(ea_t[:], ea_dram[:, g * G: (g + 1) * G])
        clr_t = sb.tile([1, G], BF16, tag="clr")
        nc.sync.dma_start(clr_t[:], clr_dram[:, g * G: (g + 1) * G])
        xw_t = sb.tile([D, 256], BF16, tag="xw")
        nc.sync.dma_start(xw_t[:], xw_dram[g * D: (g + 1) * D, :])

        # ---- x[row] features, pre-gathered on host, streamed bf16 ----
        xrT = sb.tile([D, G], BF16, tag="xrT")
        nc.sync.dma_start(xrT[:], xr_dram[:, g * G: (g + 1) * G])

        # ---- S' one-hot [node, edge] for the x[col] term ----
        sprime = sb.tile([D, G], BF16, tag="sp")
        off = 0
        for ns in subs:
            L = ns * D
            clp = pp_cl.tile([D, 512], F32, space="PSUM", tag="clp")
            nc.tensor.matmul(clp[:, 0:L], lhsT=ones_b[:],
                             rhs=clr_t[:, off: off + L], start=True, stop=True)
            nc.vector.tensor_tensor(
                out=sprime[:, off: off + L],
                in0=clp[:, 0:L],
                in1=iota_p[:].to_broadcast([D, L]),
                op=mybir.AluOpType.is_equal,
            )
            off += L

        # ---- layer 1 (feature-major), u = exp(pre + b1) in bf16 ----
        u_t = sbL.tile([D, 2 * G], BF16, tag="u")
        off = 0
        for ns in subs:
            L = ns * D
            for m in range(2):
                ms = slice(m * D, (m + 1) * D)
                pre = pp_pre.tile([D, 512], F32, space="PSUM", tag="pre")
                nc.tensor.matmul(pre[:, 0:L], lhsT=w1a[:, ms],
                                 rhs=xrT[:, off: off + L], start=True, stop=False)
                nc.tensor.matmul(pre[:, 0:L], lhsT=xw_t[:, ms],
                                 rhs=sprime[:, off: off + L], start=False, stop=False)
                nc.tensor.matmul(pre[:, 0:L], lhsT=w1c[:, ms],
                                 rhs=ea_t[:, off: off + L], start=False, stop=True)
                nc.scalar.activation(
                    u_t[:, m * G + off: m * G + off + L], pre[:, 0:L],
                    AF.Exp, bias=b1c[:, m: m + 1],
                )
            off += L
        # hT = ln(1 + u), one instruction for the whole group
        hT = sbL.tile([D, 2 * G], BF16, tag="hT")
        nc.scalar.activation(hT[:], u_t[:], AF.Ln, bias=1.0)

        # ---- layer 2 (data-stationary, edge-major out) + scatter ----
        u2 = sb.tile([D, G], BF16, tag="u2")
        v2 = sbL.tile([D, G], F32, tag="v2")
        c0 = 0
        for ns in subs:
            eps = pp_b.tile([D, 512], F32, space="PSUM", tag="eps")
            for i in range(ns):
                c = c0 + i
                es = slice(i * D, (i + 1) * D)
                nc.tensor.matmul(eps[:, es], lhsT=hT[:, c * D: (c + 1) * D],
                                 rhs=w2_0[:], start=True, stop=False)
                nc.tensor.matmul(eps[:, es],
                                 lhsT=hT[:, G + c * D: G + (c + 1) * D],
                                 rhs=w2_1[:], start=False, stop=True)
            nc.vector.tensor_add(v2[:, c0 * D: (c0 + ns) * D],
                                 eps[:, 0: ns * D], b2bc[:, 0: ns * D])
            c0 += ns
        nc.scalar.activation(u2[:], v2[:], AF.Exp)
        embs = sb.tile([D, G], BF16, tag="embs")
        nc.scalar.activation(embs[:], u2[:], AF.Ln, bias=1.0)

        agg = pp_g.tile([D, D], F32, space="PSUM", tag="agg")
        for c in range(nch):
            S_t = sb.tile([D, D], BF16, tag="S")
            nc.vector.tensor_tensor(
                out=S_t[:],
                in0=cl_t[:, g * nch + c: g * nch + c + 1].to_broadcast([D, D]),
                in1=iota_b[:],
                op=mybir.AluOpType.is_equal,
            )
            nc.tensor.matmul(agg[:], lhsT=embs[:, c * D: (c + 1) * D], rhs=S_t[:],
                             start=(c == 0), stop=(c == nch - 1))
        nc.vector.tensor_copy(aggrT[:, g * D: (g + 1) * D], agg[:])

        # ---- node MLP for finished blocks, every 4 groups (fp32) ----
        if g % 4 == 3 or g == nblk - 1:
            j0 = (g // 4) * 4
            nset = g + 1 - j0
            W = nset * D
            yps = pp_b.tile([D, 512], F32, space="PSUM", tag="eps")
            for i in range(nset):
                j = j0 + i
                ys = slice(i * D, (i + 1) * D)
                nc.tensor.matmul(yps[:, ys], lhsT=xt_t[:, j * D: (j + 1) * D],
                                 rhs=wn_x[:], start=True, stop=False)
                nc.tensor.matmul(yps[:, ys], lhsT=aggrT[:, j * D: (j + 1) * D],
                                 rhs=wn_a[:], start=False, stop=True)
            vy = sbn.tile([D, 512], F32, tag="vy")
            nc.vector.tensor_add(vy[:, 0:W], yps[:, 0:W], bnbc[:, 0:W])
            uy = sbn.tile([D, 512], F32, tag="uy")
            nc.scalar.activation(uy[:, 0:W], vy[:, 0:W], AF.Exp)
            sp = sbn.tile([D, 512], F32, tag="spn")
            nc.scalar.activation(sp[:, 0:W], uy[:, 0:W], AF.Ln, bias=1.0)
            xb_t = sbn.tile([D, 512], F32, tag="xb")
            nc.sync.dma_start(
                xb_t[:, 0:W].rearrange("p (c f) -> p c f", f=D),
                xb_dram[j0 * D: j0 * D + W, :].rearrange("(c p) f -> p c f", p=D),
            )
            ot = sbn.tile([D, 512], F32, tag="ot")
            nc.vector.tensor_add(ot[:, 0:W], sp[:, 0:W], xb_t[:, 0:W])
            nc.sync.dma_start(
                out_dram[j0 * D: j0 * D + W, :].rearrange("(c p) f -> p c f", p=D),
                ot[:, 0:W].rearrange("p (c f) -> p c f", f=D),
            )


def build_nc(nblk, nch, num_devices=1):
    nc = bacc.Bacc("TRN2", target_bir_lowering=False, debug=False,
                   num_devices=num_devices)
    G = nch * D
    GI = G // 16
    specs = {
        "xr": ([D, nblk * G], BF16),
        "b2bc": ([D, 512], F32),
        "bnbc": ([D, 512], F32),
        "xt": ([D, nblk * D], F32),
        "xb": ([nblk * D, D], F32),
        "ea": ([D, nblk * G], BF16),
        "clr": ([1, nblk * G], BF16),
        "cl": ([D, nblk * nch], BF16),
        "xw1b": ([nblk * D, 256], BF16),
        "w1a": ([D, 256], BF16),
        "w1c": ([D, 256], BF16),
        "b1c": ([D, 2], F32),
        "w2": ([256, D], BF16),
        "b2r": ([1, D], BF16),
        "wn": ([256, D], F32),
        "bnr": ([1, D], F32),
    }
    aps = {}
    for name, (shape, dt) in specs.items():
        aps[name] = nc.dram_tensor(name, shape, dt, kind="ExternalInput").ap()
    aps["out"] = nc.dram_tensor("out", [nblk * D, D], F32,
                                kind="ExternalOutput").ap()

    from contextlib import ExitStack

    with tile.TileContext(nc) as tc, ExitStack() as ctx:
        build_program(ctx, tc, aps, nblk, nch)
    nc.compile()
    return nc


def host_prep(x, edge_index, edge_attr, W1, b1, W2, b2, Wn, bn,
              n_nodes, n_cores, nblk):
    bf = ml_dtypes.bfloat16
    npc = nblk * D
    n_blocks_tot = n_cores * nblk

    row = np.asarray(edge_index[0], dtype=np.int64)
    col = np.asarray(edge_index[1], dtype=np.int64)
    E = row.shape[0]
    B = col // D
    order = np.argsort(B, kind="stable")
    counts = np.bincount(B, minlength=n_blocks_tot)
    G = int(np.ceil(max(int(counts.max()), 256) / D) * D)
    nch = G // D

    starts = np.zeros(n_blocks_tot, dtype=np.int64)
    starts[1:] = np.cumsum(counts)[:-1]
    pos = np.arange(E, dtype=np.int64) - starts[B[order]]
    slot = B[order] * G + pos            # slot in flat padded edge array

    flat_row = np.full(n_blocks_tot * G, -1, dtype=np.int64)  # -1 = padding
    flat_row[slot] = row[order]
    flat_cl = np.full(n_blocks_tot * G, 300.0, dtype=np.float32)
    flat_cl[slot] = (col[order] % D).astype(np.float32)
    flat_ea = np.zeros((n_blocks_tot * G, D), dtype=bf)
    flat_ea[slot] = edge_attr[order].astype(bf)

    x32 = np.ascontiguousarray(x).astype(np.float32)
    x_bf = x32.astype(bf)

    w1a = np.ascontiguousarray(W1[0:D]).astype(bf)
    w1b32 = np.ascontiguousarray(W1[D: 2 * D]).astype(np.float32)
    w1c = np.ascontiguousarray(W1[2 * D: 3 * D]).astype(bf)
    b1c = np.ascontiguousarray(np.asarray(b1).reshape(2, D).T).astype(np.float32)
    w2 = np.ascontiguousarray(W2).astype(bf)
    b2r = np.ascontiguousarray(np.asarray(b2)[None, :]).astype(bf)
    wn = np.ascontiguousarray(Wn).astype(np.float32)
    bnr = np.ascontiguousarray(np.asarray(bn)[None, :]).astype(np.float32)

    GI = G // 16
    in_maps = []
    for k in range(n_cores):
        lo, hi = k * npc, min((k + 1) * npc, n_nodes)
        xk = np.zeros((npc, D), dtype=np.float32)
        xk[0: hi - lo] = x32[lo:hi]

        rows_k = flat_row[k * nblk * G: (k + 1) * nblk * G]  # [nblk*G]
        xr_rows = np.zeros((nblk * G, D), dtype=bf)
        real = rows_k >= 0
        xr_rows[real] = x_bf[rows_k[real]]
        xr_k = np.ascontiguousarray(xr_rows.T)

        ea_k = np.ascontiguousarray(
            flat_ea[k * nblk * G: (k + 1) * nblk * G].T)
        cl_k = flat_cl[k * nblk * G: (k + 1) * nblk * G]
        clr = np.ascontiguousarray(cl_k[None, :]).astype(bf)
        cl_sw = np.ascontiguousarray(
            cl_k.reshape(nblk, nch, D).transpose(2, 0, 1).reshape(D, nblk * nch)
        ).astype(bf)
        xw1b = (xk @ w1b32).astype(bf)   # [npc, 256]

        in_maps.append({
            "xr": xr_k,
            "b2bc": np.tile(np.asarray(b2, np.float32)[None, :], (D, 4)),
            "bnbc": np.tile(np.asarray(bn, np.float32)[None, :], (D, 4)),
            "xt": np.ascontiguousarray(xk.T), "xb": xk,
            "ea": ea_k, "clr": clr, "cl": cl_sw,
            "xw1b": np.ascontiguousarray(xw1b),
            "w1a": w1a, "w1c": w1c, "b1c": b1c,
            "w2": w2, "b2r": b2r, "wn": wn, "bnr": bnr,
        })
    return in_maps, nch


def run(inputs, trace=False, **kw):
    in_maps, nch = host_prep(
        inputs["x"], inputs["edge_index"], inputs["edge_attr"],
        inputs["W1"], inputs["b1"], inputs["W2"], inputs["b2"],
        inputs["Wn"], inputs["bn"],
        n_nodes=N_NODES, n_cores=N_CORES, nblk=NBLK,
    )
    nc = build_nc(NBLK, nch, num_devices=N_CORES)
    res = run_bass_kernel_spmd(nc, in_maps, core_ids=list(range(N_CORES)),
                               trace=trace, **kw)
    out = np.concatenate([res.results[k]["out"] for k in range(N_CORES)], axis=0)
    return out[:N_NODES], res


def kernel(**inputs) -> np.ndarray:
    out, _ = run(inputs, trace=False)
    return np.ascontiguousarray(out.astype(np.float32))
